# revision 27
# baseline (speedup 1.0000x reference)
"""2-layer GAT on 8 Trainium2 NeuronCores (Bass/Tile).

Strategy (dst-sharded graph parallelism):
  - Layer-0 dense part (h = feat @ W0, plus fused attention-logit columns
    el = (h*al).sum(-1), er = (h*ar).sum(-1) via host-precomputed extra weight
    columns) is computed REPLICATED on every core into local DRAM tables
    (collectives are slow, ~62 GB/s, so replicating the cheap dense compute
    beats an AllGather of h).
  - Edges are sorted by destination on the host and sharded by dst-node range
    (6250 dst nodes per core).  Edge blocks of 128 land on SBUF partitions.
  - Per-edge source rows are fetched with dma_gather (int16 indices).  Since
    int16 only addresses 32768 rows, the node table is stored twice: window A
    = conceptual rows [0, 32768) and window B = conceptual rows [17234, 50002)
    where conceptual row 0 and row 50001 are zero guard rows and node i lives
    at conceptual row i+1.  Each edge block is single-window by construction
    (host splits each dst tile's edges into lo/hi runs, padded to x128).
  - er values (indexed by dst) are first compacted into a core-local table
    erloc[local_dst] via two window-gathers + add (invalid side hits a zero
    guard row), then per-edge er comes from a dma_gather on erloc with local
    (< 6250, int16-safe) indices.
  - Per edge block: expe = exp(max(x, 0.2x)) where x = el[src]+er[dst], a 0/1
    selection matrix mask[e, j] = (dstloc[e]==j) via DVE is_equal against an
    iota row, and one fp32 matmul per block accumulates BOTH the weighted
    message sum and the softmax denominator into PSUM:
        psum[j, 0:256] += sum_e mask[e,j] * (expe[e,h] * h[src_e])
        psum[j, 256:260] += sum_e mask[e,j] * expe[e,h]
  - Finalize divides by the denominator, runs layer-1's dense projection on
    the local dst slice, AllGathers the small [N, 42] projected table
    (8.4 MB), repacks it into window tables, and repeats the edge pipeline
    for layer 1 to produce the logits slice per core (host concatenates).

The edge structure is computed at runtime from the actual inputs and padded
to a uniform shape across cores (SPMD = one program for all 8 cores).
"""

import os
import numpy as np

import concourse.bass as bass
import concourse.bacc as bacc
import concourse.mybir as mybir
import concourse.tile as tile
from concourse.bass_utils import run_bass_kernel_spmd

F32 = mybir.dt.float32
I16 = mybir.dt.int16

SLOPE = 0.2
NCORES = 8
P = 128
G = 16          # max edge blocks per gather chunk
ST = 4          # dst tiles per supertile (lo/hi run batching)
CH = 8          # node tiles per phase-A chunk
WROWS = 32768   # rows per index window
LAST_EXEC_NS = [None]
LAST_RES = [None]
LAST_SIM = [None]
LAST_BUILD = [None]


def _bcast_inner(apv, count):
    return bass.AP(tensor=apv.tensor, offset=apv.offset, ap=apv.ap + [[0, count]])


def _bcast_mid(apv, count):
    a = apv.ap
    return bass.AP(tensor=apv.tensor, offset=apv.offset, ap=[a[0], [0, count]] + a[1:])


def _fuse_w(W, al, ar):
    Fin = W.shape[0]
    H, D = al.shape
    Wr = W.reshape(Fin, H, D)
    wl = np.einsum("khd,hd->kh", Wr, al).astype(np.float32)
    wr = np.einsum("khd,hd->kh", Wr, ar).astype(np.float32)
    return np.ascontiguousarray(np.concatenate([W, wl, wr], axis=1), dtype=np.float32)


def _wrap16(idx):
    """int16 idx list (len multiple of 128) -> dma_gather SBUF layout
    [128, len/16]: idx j at [j % 16, j // 16], replicated across 8 groups."""
    w = idx.reshape(-1, 16).T.astype(np.int16)
    return np.ascontiguousarray(np.tile(w, (8, 1)))


def _prep_edges(src, dst, n_nodes, ncores, wrows):
    from types import SimpleNamespace
    plan = SimpleNamespace()
    npc = n_nodes // ncores
    tpc = (npc + P - 1) // P
    plan.npc, plan.tpc = npc, tpc
    wa_max = wrows - 2               # node i valid in A iff i+1 <= wrows-1
    plan.wb_base = n_nodes + 2 - wrows

    order = np.argsort(dst, kind="stable")
    ss = src[order].astype(np.int64)
    ds = dst[order].astype(np.int64)
    core = ds // npc
    loc = ds % npc
    tileid = loc // P
    hi = (ss > wa_max).astype(np.int64)

    counts = np.zeros((ncores, tpc, 2), np.int64)
    np.add.at(counts, (core, tileid, hi), 1)
    nblk = (counts + P - 1) // P
    bcnt = nblk.max(axis=0)
    if bcnt.sum() == 0:
        bcnt[0, 0] = 1
    plan.bcnt = bcnt

    plan.sts = [list(range(s, min(s + ST, tpc))) for s in range(0, tpc, ST)]
    plan.order_blocks = []
    for tiles in plan.sts:
        for w in (0, 1):
            for t in tiles:
                plan.order_blocks += [(t, w)] * int(bcnt[t, w])
    plan.totblk = len(plan.order_blocks)
    plan.nedge = plan.totblk * P

    slot = {}
    pos = 0
    for (t, w) in plan.order_blocks:
        if (t, w) not in slot:
            slot[(t, w)] = pos
        pos += P

    srcw = np.zeros((ncores, plan.nedge), np.int64)
    erw = np.zeros((ncores, plan.nedge), np.int64)
    dstloc = np.full((ncores, plan.nedge), 999.0, np.float32)
    for bi, (t, w) in enumerate(plan.order_blocks):
        if w == 1:
            srcw[:, bi * P:(bi + 1) * P] = wrows - 1

    # order edges by (core, tile, win) groups
    gkey = (core * tpc + tileid) * 2 + hi
    g_order = np.argsort(gkey, kind="stable")
    ss2, loc2, gkey2 = ss[g_order], loc[g_order], gkey[g_order]
    gstart = np.zeros(ncores * tpc * 2 + 1, np.int64)
    np.add.at(gstart[1:], gkey2, 1)
    gstart = np.cumsum(gstart)
    for c in range(ncores):
        for t in range(tpc):
            for w in (0, 1):
                k = (c * tpc + t) * 2 + w
                e0, e1 = int(gstart[k]), int(gstart[k + 1])
                cnt = e1 - e0
                if cnt == 0:
                    continue
                off = slot[(t, w)]
                srcs = ss2[e0:e1]
                srcw[c, off:off + cnt] = (
                    srcs + 1 if w == 0 else srcs + 1 - plan.wb_base)
                erw[c, off:off + cnt] = loc2[e0:e1]
                dstloc[c, off:off + cnt] = (loc2[e0:e1] % P).astype(np.float32)

    plan.srcw, plan.erw, plan.dstlocv = srcw, erw, dstloc
    return plan


def _edge_phase(nc, tc, pools, tabA_ap, tabB_ap, erloc_ap, row_w, er_off, nheads,
                hdim, plan, src16_sb, er16_sb, dstloc_sb, iota_sb, gw, finalize):
    """Edge pipeline for one layer.  Gathered row: [h | el | ...], gw elems
    (multiple of 64 f32).  er gathered from erloc rows (er value at er_off)."""
    d = nheads * hdim
    hg_pool, ms_pool, mask_pool, small_pool, psum_pool = pools
    # per-tile first/last block ids
    first_blk, last_blk = {}, {}
    for bi, (t, w) in enumerate(plan.order_blocks):
        if t not in first_blk:
            first_blk[t] = bi
        last_blk[t] = bi
    acc_by_tile = {}

    # chunks: maximal runs of <=G blocks within a single window
    chunks = []
    cur = None
    for bi, (t, w) in enumerate(plan.order_blocks):
        if cur is None or cur[0] != w or bi - cur[1] >= G:
            if cur is not None:
                chunks.append(cur)
            cur = [w, bi, bi + 1]
        else:
            cur[2] = bi + 1
        if cur[2] - cur[1] >= G:
            chunks.append(cur)
            cur = None
    if cur is not None:
        chunks.append(cur)

    for w, b0, b1 in chunks:
        nb = b1 - b0
        nidx = nb * P
        HG = hg_pool.tile([P, G, gw], F32, tag="hg", name="hg")
        nc.gpsimd.dma_gather(
            out_ap=HG[:, :nb, :], in_ap=(tabA_ap if w == 0 else tabB_ap),
            idxs_ap=src16_sb[:, b0 * 8:b1 * 8], num_idxs=nidx,
            num_idxs_reg=nidx, elem_size=gw, elem_step=row_w,
            single_packet=False)
        ERG = small_pool.tile([P, G, 64], F32, tag="erg", name="erg")
        nc.gpsimd.dma_gather(
            out_ap=ERG[:, :nb, :], in_ap=erloc_ap,
            idxs_ap=er16_sb[:, b0 * 8:b1 * 8], num_idxs=nidx,
            num_idxs_reg=nidx, elem_size=64, elem_step=64,
            single_packet=False)
        # expe = exp(max(x, slope*x)), x = el + er
        E4 = small_pool.tile([P, G, nheads], F32, tag="e4", name="e4")
        nc.vector.tensor_add(E4[:, :nb, :], HG[:, :nb, d:d + nheads],
                             ERG[:, :nb, er_off:er_off + nheads])
        ESC = small_pool.tile([P, G, nheads], F32, tag="esc", name="esc")
        nc.vector.tensor_scalar_mul(ESC[:, :nb, :], E4[:, :nb, :], SLOPE)
        nc.vector.tensor_tensor(out=E4[:, :nb, :], in0=E4[:, :nb, :],
                                in1=ESC[:, :nb, :], op=mybir.AluOpType.max)
        nc.scalar.activation(out=E4[:, :nb, :], in_=E4[:, :nb, :],
                             func=mybir.ActivationFunctionType.Exp)
        # mask[p, b, j] = (dstloc[p, b] == j)
        MASK = mask_pool.tile([P, G, P], F32, tag="mask", name="mask")
        nc.vector.tensor_tensor(
            out=MASK[:, :nb, :],
            in0=_bcast_inner(dstloc_sb[:, b0:b1], P),
            in1=_bcast_mid(iota_sb[:], nb),
            op=mybir.AluOpType.is_equal)
        # MS = [expe-scaled h | expe]
        msw = d + nheads
        MS = ms_pool.tile([P, G, msw], F32, tag="ms", name="ms")
        for h in range(nheads):
            nc.vector.tensor_tensor(
                out=MS[:, :nb, h * hdim:(h + 1) * hdim],
                in0=HG[:, :nb, h * hdim:(h + 1) * hdim],
                in1=_bcast_inner(E4[:, :nb, h:h + 1], hdim),
                op=mybir.AluOpType.mult)
        nc.scalar.copy(out=MS[:, :nb, d:d + nheads], in_=E4[:, :nb, :])
        for bi in range(b0, b1):
            t, _ = plan.order_blocks[bi]
            if bi == first_blk[t]:
                acc_by_tile[t] = psum_pool.tile([P, msw], F32, tag="acc",
                                                name="acc")
            acc = acc_by_tile[t]
            nc.tensor.matmul(acc[:], lhsT=MASK[:, bi - b0, :],
                             rhs=MS[:, bi - b0, :],
                             start=(bi == first_blk[t]),
                             stop=(bi == last_blk[t]))
            if bi == last_blk[t]:
                finalize(t, acc)
                del acc_by_tile[t]


def build_and_run(feat, src, dst, W0, al0, ar0, W1, al1, ar1, trace=False,
                  simulate=False):
    n_nodes = feat.shape[0]
    npc = n_nodes // NCORES
    nh0 = al0.shape[0]
    hid0 = al0.shape[1]
    d0 = nh0 * hid0                        # 256
    row0 = ((d0 + 2 * nh0 + 63) // 64) * 64  # 320 f32 = 1280B (x256B ok)
    nh1 = al1.shape[0]
    hid1 = al1.shape[1]
    d1 = nh1 * hid1                        # 40
    row1 = max(((d1 + 2 * nh1 + 63) // 64) * 64, 128)  # 128 f32 = 512B rows
    gw1 = row1
    in_dim = feat.shape[1]
    wrows = min(WROWS, n_nodes + 2)
    wb_base = n_nodes + 2 - wrows

    w0e = _fuse_w(W0, al0, ar0)            # [in_dim, d0+2nh0]
    w1e = _fuse_w(W1, al1, ar1)            # [d0, d1+2nh1]
    kchunks = d0 // P
    w1p = np.ascontiguousarray(
        w1e.reshape(kchunks, P, d1 + 2 * nh1).transpose(1, 0, 2))

    plan = _prep_edges(src, dst, n_nodes, NCORES, wrows)
    totblk = plan.totblk
    tpc = plan.tpc
    tpc_out = tpc

    # erloc build index lists (local node -> window row or zero guard)
    gidx = np.arange(npc, dtype=np.int64)
    npc_pad = ((npc + P - 1) // P) * P
    bia = np.zeros((NCORES, npc_pad), np.int64)
    bib = np.full((NCORES, npc_pad), wrows - 1, np.int64)
    for c in range(NCORES):
        g = c * npc + gidx
        a_ok = g + 1 <= wrows - 1
        bia[c, :npc] = np.where(a_ok, g + 1, 0)
        bib[c, :npc] = np.where(~a_ok, g + 1 - wb_base, wrows - 1)

    iota = np.broadcast_to(np.arange(P, dtype=np.float32), (P, P)).copy()
    ident = np.eye(P, dtype=np.float32)

    nc = bacc.Bacc(None, target_bir_lowering=False, num_devices=NCORES)
    feat_t = nc.declare_dram_parameter("feat", [n_nodes, in_dim], F32, False)
    w0e_t = nc.declare_dram_parameter("w0e", [in_dim, d0 + 2 * nh0], F32, False)
    w1e_t = nc.declare_dram_parameter("w1e", [P, kchunks, d1 + 2 * nh1], F32, False)
    iota_t = nc.declare_dram_parameter("iota", [P, P], F32, False)
    ident_t = nc.declare_dram_parameter("ident", [P, P], F32, False)
    src16_t = nc.declare_dram_parameter("src16", [P, totblk * 8], I16, False)
    er16_t = nc.declare_dram_parameter("er16", [P, totblk * 8], I16, False)
    dstloc_t = nc.declare_dram_parameter("dstloc", [P, totblk], F32, False)
    bia_t = nc.declare_dram_parameter("bia16", [P, npc_pad // 16], I16, False)
    bib_t = nc.declare_dram_parameter("bib16", [P, npc_pad // 16], I16, False)
    out_t = nc.declare_dram_parameter("out", [npc, d1], F32, True)

    tab0A = nc.dram_tensor("tab0A", [wrows, row0], F32)
    tab0B = nc.dram_tensor("tab0B", [wrows, row0], F32)
    tab1A = nc.dram_tensor("tab1A", [wrows, row1], F32)
    tab1B = nc.dram_tensor("tab1B", [wrows, row1], F32)
    erloc0 = nc.dram_tensor("erloc0", [npc_pad, 64], F32)
    erloc1 = nc.dram_tensor("erloc1", [npc_pad, 64], F32)
    h2slice = nc.dram_tensor("h2slice", [npc, d1 + 2 * nh1], F32)
    h2full = nc.dram_tensor("h2full", [NCORES, npc, d1 + 2 * nh1], F32,
                            addr_space="Shared")

    debug = os.environ.get("GAT_DEBUG", "0") == "1"
    phases = os.environ.get("GAT_PHASES", "full")
    if debug:
        dbg_t = {
            "tab0A": nc.declare_dram_parameter("dbg_tab0A", [wrows, row0], F32, True),
            "erloc0": nc.declare_dram_parameter("dbg_erloc0", [npc_pad, 64], F32, True),
            "h2s": nc.declare_dram_parameter("dbg_h2s", [npc, d1 + 2 * nh1], F32, True),
            "erloc1": nc.declare_dram_parameter("dbg_erloc1", [npc_pad, 64], F32, True),
        }

    nt_full = n_nodes // P
    rem = n_nodes - nt_full * P
    # phase-A window write ranges (node index ranges)
    wa_nodes = (0, wrows - 1)
    wb_nodes = (wb_base - 1, n_nodes)  # nodes wb_base-1 .. -> tabB rows i+1-wb_base

    with tile.TileContext(nc) as tc:
        with tc.tile_pool(name="singles", bufs=1) as singles:
            iota_sb = singles.tile([P, P], F32)
            nc.sync.dma_start(out=iota_sb[:], in_=iota_t.ap())
            ident_sb = singles.tile([P, P], F32)
            nc.sync.dma_start(out=ident_sb[:], in_=ident_t.ap())
            w0e_sb = singles.tile([P, d0 + 2 * nh0], F32)
            nc.sync.dma_start(out=w0e_sb[:], in_=w0e_t.ap())
            w1e_sb = singles.tile([P, kchunks, d1 + 2 * nh1], F32)
            nc.sync.dma_start(out=w1e_sb[:], in_=w1e_t.ap())
            src16_sb = singles.tile([P, totblk * 8], I16)
            nc.sync.dma_start(out=src16_sb[:], in_=src16_t.ap())
            er16_sb = singles.tile([P, totblk * 8], I16)
            nc.sync.dma_start(out=er16_sb[:], in_=er16_t.ap())
            dstloc_sb = singles.tile([P, totblk], F32)
            nc.sync.dma_start(out=dstloc_sb[:], in_=dstloc_t.ap())
            bia_sb = singles.tile([P, npc_pad // 16], I16)
            nc.sync.dma_start(out=bia_sb[:], in_=bia_t.ap())
            bib_sb = singles.tile([P, npc_pad // 16], I16)
            nc.sync.dma_start(out=bib_sb[:], in_=bib_t.ap())
            zrow = singles.tile([P, row0], F32)
            nc.vector.memset(zrow[:], 0.0)
            # zero guard rows
            nc.sync.dma_start(out=tab0A.ap()[0:1], in_=zrow[:1, :row0])
            nc.sync.dma_start(out=tab0B.ap()[wrows - 1:wrows], in_=zrow[:1, :row0])
            nc.sync.dma_start(out=tab1A.ap()[0:1], in_=zrow[:1, :row1])
            nc.sync.dma_start(out=tab1B.ap()[wrows - 1:wrows], in_=zrow[:1, :row1])

            # ---- Phase A: replicated dense layer 0 -> tab0A/tab0B ----
            with (tc.tile_pool(name="pa", bufs=2) as pa,
                  tc.tile_pool(name="pa_fts", bufs=3) as pa_fts,
                  tc.tile_pool(name="pa_ps", bufs=2, space="PSUM") as pa_ps,
                  tc.tile_pool(name="pa_ph", bufs=2, space="PSUM") as pa_ph):
                base = 0
                while base < n_nodes:
                    ch = min(CH, (n_nodes - base) // P)
                    partial = ch == 0
                    ch = max(ch, 1)
                    rows = rem if partial else ch * P
                    fchunk = pa.tile([P, CH, in_dim], F32, tag="fchunk",
                                     name="fchunk")
                    if partial:
                        nc.vector.memset(fchunk[:, 0, :], 0.0)
                        nc.sync.dma_start(out=fchunk[:rows, 0, :],
                                          in_=feat_t.ap()[base:base + rows])
                    else:
                        nc.sync.dma_start(
                            out=fchunk[:, :ch, :],
                            in_=feat_t.ap()[base:base + rows].rearrange(
                                "(i p) d -> p i d", p=P))
                    hstage = pa.tile([P, CH, row0], F32, tag="hstage",
                                     name="hstage")
                    if row0 > d0 + 2 * nh0:
                        nc.vector.memset(hstage[:, :, d0 + 2 * nh0:row0], 0.0)
                    for i in range(ch):
                        ftp = pa_ps.tile([P, P], F32, name="ftp")
                        nc.tensor.transpose(ftp[:], fchunk[:, i, :], ident_sb[:])
                        fts = pa_fts.tile([P, P], F32, name="fts")
                        nc.scalar.copy(out=fts[:], in_=ftp[:])
                        hps = pa_ph.tile([P, d0 + 2 * nh0], F32, name="hps")
                        nc.tensor.matmul(hps[:], lhsT=fts[:], rhs=w0e_sb[:],
                                         start=True, stop=True)
                        nc.scalar.copy(out=hstage[:, i, 0:d0 + 2 * nh0],
                                       in_=hps[:])
                    # write chunk rows [base, base+rows) to each window table
                    vw = row0
                    for (tab, lo_n, hi_n, roff) in (
                            (tab0A, wa_nodes[0], wa_nodes[1], 1),
                            (tab0B, wb_nodes[0], wb_nodes[1], 1 - wb_base)):
                        lo = max(base, lo_n)
                        hi = min(base + rows, hi_n)
                        if lo >= hi:
                            continue
                        if partial:
                            nc.sync.dma_start(
                                out=tab.ap()[lo + roff:hi + roff, 0:vw],
                                in_=hstage[lo - base:hi - base, 0, 0:vw])
                        elif lo == base and hi == base + rows:
                            nc.sync.dma_start(
                                out=tab.ap()[lo + roff:hi + roff, 0:vw].rearrange(
                                    "(i p) d -> p i d", p=P),
                                in_=hstage[:, :ch, 0:vw])
                        else:
                            for i in range(ch):
                                t0 = base + i * P
                                l2, h2 = max(lo, t0), min(hi, t0 + P)
                                if l2 >= h2:
                                    continue
                                nc.sync.dma_start(
                                    out=tab.ap()[l2 + roff:h2 + roff, 0:vw],
                                    in_=hstage[l2 - t0:h2 - t0, i, 0:vw])
                    base += rows

            # ---- shared pools for edge phases ----
            with (tc.tile_pool(name="hg", bufs=2) as hg_pool,
                  tc.tile_pool(name="ms", bufs=2) as ms_pool,
                  tc.tile_pool(name="mk", bufs=2) as mask_pool,
                  tc.tile_pool(name="sm", bufs=3) as small_pool,
                  tc.tile_pool(name="fin", bufs=2) as fin_pool,
                  tc.tile_pool(name="ps_acc", bufs=5, space="PSUM") as psum_pool,
                  tc.tile_pool(name="ps_tp", bufs=2, space="PSUM") as psum_tp,
                  tc.tile_pool(name="ps_h2", bufs=1, space="PSUM") as psum_h2):

                def build_erloc(tabA, tabB, erloc, width, col0):
                    nseg = npc_pad // P
                    with tc.tile_pool(name="ebld", bufs=1) as ebld:
                        EA = ebld.tile([P, nseg, 64], F32, tag="erga", name="ea")
                        nc.gpsimd.dma_gather(
                            out_ap=EA[:], in_ap=tabA.ap()[:, col0:col0 + 64],
                            idxs_ap=bia_sb[:], num_idxs=npc_pad,
                            num_idxs_reg=npc_pad, elem_size=64, elem_step=width,
                            single_packet=False)
                        EB = ebld.tile([P, nseg, 64], F32, tag="ergb", name="eb")
                        nc.gpsimd.dma_gather(
                            out_ap=EB[:], in_ap=tabB.ap()[:, col0:col0 + 64],
                            idxs_ap=bib_sb[:], num_idxs=npc_pad,
                            num_idxs_reg=npc_pad, elem_size=64, elem_step=width,
                            single_packet=False)
                        nc.vector.tensor_add(EA[:], EA[:], EB[:])
                        nc.sync.dma_start(
                            out=erloc.ap().rearrange("(i p) d -> p i d", p=P),
                            in_=EA[:])

                # erloc0: er at table cols [260:264] -> stored col 4+256-260...
                # gather window [row0-64, row0) covers cols 256:320; er is at
                # cols 260:264 -> offset 4 within the gathered 64
                if phases != "a":
                    build_erloc(tab0A, tab0B, erloc0, row0, row0 - 64)
                er_off0 = (d0 + nh0) - (row0 - 64)   # = 260-256 = 4

                def finalize0(t, acc):
                    rows = min(P, npc - t * P)
                    S = small_pool.tile([P, nh0], F32, tag="s0", name="s0")
                    nc.vector.tensor_scalar_max(S[:], acc[:, d0:d0 + nh0], 1e-30)
                    RC = small_pool.tile([P, nh0], F32, tag="rc0", name="rc0")
                    nc.vector.reciprocal(RC[:], S[:])
                    H1T = fin_pool.tile([P, d0], F32, tag="h1t", name="h1t")
                    nc.vector.tensor_tensor(
                        out=H1T[:].rearrange("p (h e) -> p h e", h=nh0),
                        in0=acc[:, 0:d0].rearrange("p (h e) -> p h e", h=nh0),
                        in1=_bcast_inner(RC[:], hid0),
                        op=mybir.AluOpType.mult)
                    h2ps = psum_h2.tile([P, d1 + 2 * nh1], F32, name="h2ps")
                    for k in range(kchunks):
                        tp = psum_tp.tile([P, P], F32, name="tp")
                        nc.tensor.transpose(tp[:], H1T[:, k * P:(k + 1) * P],
                                            ident_sb[:])
                        ts = fin_pool.tile([P, P], F32, tag="tsb", name="tsb")
                        nc.scalar.copy(out=ts[:], in_=tp[:])
                        nc.tensor.matmul(h2ps[:], lhsT=ts[:], rhs=w1e_sb[:, k, :],
                                         start=(k == 0), stop=(k == kchunks - 1))
                    h2sb = fin_pool.tile([P, d1 + 2 * nh1], F32, tag="h2sb",
                                         name="h2sb")
                    nc.scalar.copy(out=h2sb[:], in_=h2ps[:])
                    nc.sync.dma_start(out=h2slice.ap()[t * P:t * P + rows],
                                      in_=h2sb[:rows, :])

                if phases != "a":
                    _edge_phase(nc, tc,
                                (hg_pool, ms_pool, mask_pool, small_pool,
                                 psum_pool),
                                tab0A.ap(), tab0B.ap(), erloc0.ap(), row0,
                                er_off0, nh0, hid0, plan, src16_sb, er16_sb,
                                dstloc_sb, iota_sb, row0, finalize0)

                # ---- AllGather projected table, repack into window tables ----
                run_l1 = phases in ("full", "abc")
                if run_l1:
                    nc.gpsimd.collective_compute(
                    "AllGather", mybir.AluOpType.bypass,
                        replica_groups=[list(range(NCORES))],
                        ins=[h2slice.ap()], outs=[h2full.ap()])
                    h2flat = h2full.ap().rearrange("c n d -> (c n) d")
                    rw1 = d1 + 2 * nh1
                    na = min(wrows - 2, n_nodes - 1) + 1
                    nc.sync.dma_start(out=tab1A.ap()[1:1 + na, 0:rw1],
                                      in_=h2flat[0:na])
                    blo = max(wb_base - 1, 0)
                    nc.sync.dma_start(
                        out=tab1B.ap()[blo + 1 - wb_base:n_nodes + 1 - wb_base,
                                       0:rw1],
                        in_=h2flat[blo:n_nodes])
                    build_erloc(tab1A, tab1B, erloc1, row1, 0)
                er_off1 = d1 + nh1   # er-build window starts at col 0

                def finalize1(t, acc):  # noqa: indent-kept
                    rows = min(P, npc - t * P)
                    S = small_pool.tile([P, nh1], F32, tag="s1", name="s1")
                    nc.vector.tensor_scalar_max(S[:], acc[:, d1:d1 + nh1], 1e-30)
                    RC = small_pool.tile([P, nh1], F32, tag="rc1", name="rc1")
                    nc.vector.reciprocal(RC[:], S[:])
                    OUT = fin_pool.tile([P, d1], F32, tag="outt", name="outt")
                    nc.vector.tensor_scalar_mul(OUT[:], acc[:, 0:d1], RC[:, 0:1])
                    nc.sync.dma_start(out=out_t.ap()[t * P:t * P + rows],
                                      in_=OUT[:rows, :])

                if phases == "full":
                    _edge_phase(nc, tc,
                                (hg_pool, ms_pool, mask_pool, small_pool,
                                 psum_pool),
                                tab1A.ap(), tab1B.ap(), erloc1.ap(), row1,
                                er_off1, nh1, hid1, plan, src16_sb, er16_sb,
                                dstloc_sb, iota_sb, row1, finalize1)
                else:
                    ztile = fin_pool.tile([P, d1], F32, tag="outt", name="zout")
                    nc.vector.memset(ztile[:], 0.0)
                    for t in range(tpc_out):
                        rows = min(P, npc - t * P)
                        nc.sync.dma_start(out=out_t.ap()[t * P:t * P + rows],
                                          in_=ztile[:rows, :])

                if debug:
                    nc.sync.dma_start(out=dbg_t["tab0A"].ap(), in_=tab0A.ap())
                    nc.sync.dma_start(out=dbg_t["erloc0"].ap(), in_=erloc0.ap())
                    nc.sync.dma_start(out=dbg_t["h2s"].ap(), in_=h2slice.ap())
                    nc.sync.dma_start(out=dbg_t["erloc1"].ap(), in_=erloc1.ap())

    nc.compile()

    in_maps = []
    for c in range(NCORES):
        in_maps.append({
            "feat": np.ascontiguousarray(feat, dtype=np.float32),
            "w0e": w0e,
            "w1e": w1p,
            "iota": iota,
            "ident": ident,
            "src16": _wrap16(plan.srcw[c]),
            "er16": _wrap16(plan.erw[c]),
            "dstloc": np.ascontiguousarray(
                plan.dstlocv[c].reshape(totblk, P).T.astype(np.float32)),
            "bia16": np.ascontiguousarray(
                np.tile(bia[c].reshape(-1, 16).T.astype(np.int16), (8, 1))),
            "bib16": np.ascontiguousarray(
                np.tile(bib[c].reshape(-1, 16).T.astype(np.int16), (8, 1))),
        })
    LAST_BUILD[0] = (nc, in_maps)
    if simulate:
        from concourse import bass_interp
        sim = bass_interp.MultiCoreSim(nc, NCORES, ignore_data_errors=True)
        for c in range(NCORES):
            for k, v in in_maps[c].items():
                sim.cores[c].tensor(k)[:] = v
        sim.simulate()
        LAST_SIM[0] = sim
        out = np.concatenate(
            [np.array(sim.cores[c].tensor("out")) for c in range(NCORES)], axis=0)
        return out
    res = run_bass_kernel_spmd(nc, in_maps, list(range(NCORES)), trace=trace)
    LAST_RES[0] = res
    LAST_EXEC_NS[0] = res.exec_time_ns
    out = np.concatenate([res.results[c]["out"] for c in range(NCORES)], axis=0)
    return out


def kernel(feat, src, dst, W0, al0, ar0, W1, al1, ar1):
    trace = os.environ.get("GAT_TRACE", "0") == "1"
    out = build_and_run(np.asarray(feat), np.asarray(src), np.asarray(dst),
                        np.asarray(W0), np.asarray(al0), np.asarray(ar0),
                        np.asarray(W1), np.asarray(al1), np.asarray(ar1),
                        trace=trace)
    return out.astype(np.float32)
